# revision 19
# baseline (speedup 1.0000x reference)
"""GCNConv message-passing kernel for 8 Trainium2 NeuronCores.

Strategy (edge/graph parallelism, sharded by destination row):
  - 50000 rows are dealt into 392 blocks (8 cores x 49) by a degree
    balancer so every block holds <= 128 rows and <= 4096 edges; each
    block is exactly K=32 chunks of 128 edges (KL=16 lo + KH=16 hi).
  - the gather table is split into two OVERLAPPING halves
    xlo=x[0:32768], xhi=x[17232:50000] so cols in [17232,32768) can be
    assigned to either side ("flex"), letting every block fill exactly
    KL lo-chunks and KH hi-chunks with no split padding.
  - on device, per chunk: SWDGE dma_gather fetches x[col] rows (bf16),
    DVE builds a norm-scaled one-hot P[e,r] = norm_e * (row_rel_e == r),
    PE accumulates psum_x[r,:] += P^T @ x_g and psum_e[r,:] += P^T @ ea
  - per block: transpose agg, apply W (bf16), add bias, DMA out
  - edge_attr streamed as fp8e4m3, output stored bf16 (CPU upcasts)
  - all gather index tiles are DMAed up front so SWDGE descriptor
    generation never waits behind the big gather transfers; one lo and
    one hi dma_gather call per block so a block's compute gates only on
    its own slice of the gather stream
  - x-side scatter emits agg^T directly (lhsT=x_g, rhs=P), ea-side uses
    the cheap orientation (out free dim 32) plus one PE transpose
  - a per-block dummy DVE read of the 2-blocks-ago PSUM tile emits one
    dominating cross-engine wait so the per-chunk P anti-dep waits are
    elided (DVE SEQ cadence 140ns -> 70ns per chunk)
  - giter sizes (4,...,4,2,2,2,1,1,1): small tail giters so little
    compute trails the final gather DMA
  - no collectives needed (cores own disjoint output rows)
"""
import sys
import numpy as np
import ml_dtypes

for _p in ("/opt/trn_rl_repo", "/root/.axon_site/_ro/trn_rl_repo"):
    if _p not in sys.path:
        sys.path.insert(0, _p)

N_NODES = 50000
N_EDGES = 1600000
IN_CH = 128
EDGE_DIM = 32
OUT_CH = 128
F = IN_CH + EDGE_DIM            # 160
N_CORES = 8
BLK = 128
NB = 49                         # blocks per core
NBLOCKS = N_CORES * NB          # 392
SLOTS = NB * BLK                # 6272 output slots per core
SPLIT = 32768                   # int16 gather index limit
HI_BASE = N_NODES - SPLIT       # 17232: xhi = x[HI_BASE:]
GITERS = (4,) * 10 + (2, 2, 2, 1, 1, 1)  # blocks per giter (sum = 49); small tail
CPC = 64                        # chunks per dma_gather call (8192-idx cap)

_NC_CACHE = {}


def _to_bf16(a):
    """fast f32 -> bf16 with round-to-nearest-ish."""
    u = np.ascontiguousarray(a, dtype=np.float32).view(np.uint32)
    return ((u + 0x8000) >> 16).astype(np.uint16).view(ml_dtypes.bfloat16)


def _build_nc(KL, KH, skip=()):
    from concourse import bacc, mybir
    from concourse.tile import TileContext

    K = KL + KH
    BF16 = mybir.dt.bfloat16
    F32 = mybir.dt.float32
    FP8 = mybir.dt.float8e4
    I16 = mybir.dt.int16

    nc = bacc.Bacc(None, target_bir_lowering=False, num_swdge_queues=4)
    xlo = nc.dram_tensor("xlo", [SPLIT, IN_CH], BF16, kind="ExternalInput")
    xhi = nc.dram_tensor("xhi", [N_NODES - HI_BASE, IN_CH], BF16,
                         kind="ExternalInput")
    idxlo = nc.dram_tensor("idxlo", [128, NB * KL * 8], I16, kind="ExternalInput")
    idxhi = nc.dram_tensor("idxhi", [128, NB * KH * 8], I16, kind="ExternalInput")
    NGI = len(GITERS)
    # partition-major streams over the whole core: [p, b*K + k, :]
    ea_h = nc.dram_tensor("ea_h", [128, NB * K, EDGE_DIM], FP8,
                          kind="ExternalInput")
    rr_h = nc.dram_tensor("rr_h", [128, NB * K, 2], F32, kind="ExternalInput")
    iota_h = nc.dram_tensor("iota_h", [128, 128], BF16, kind="ExternalInput")
    ident_h = nc.dram_tensor("ident_h", [128, 128], BF16, kind="ExternalInput")
    W_h = nc.dram_tensor("W_h", [F, OUT_CH], BF16, kind="ExternalInput")
    b_h = nc.dram_tensor("b_h", [1, OUT_CH], BF16, kind="ExternalInput")
    ones_h = nc.dram_tensor("ones_h", [1, 128], BF16, kind="ExternalInput")
    out = nc.dram_tensor("out", [SLOTS, OUT_CH], BF16, kind="ExternalOutput")

    with TileContext(nc) as tc:
        with tc.tile_pool(name="const", bufs=1) as cp, \
             tc.tile_pool(name="gidx", bufs=1) as gip, \
             tc.tile_pool(name="gbuf", bufs=3) as gp, \
             tc.tile_pool(name="ebuf", bufs=3) as ep, \
             tc.tile_pool(name="pbuf", bufs=64) as pb, \
             tc.tile_pool(name="stage", bufs=2) as st, \
             tc.tile_pool(name="agg", bufs=2, space="PSUM") as pagg, \
             tc.tile_pool(name="misc", bufs=1, space="PSUM") as pmisc:
            # gather index tiles FIRST (per-giter tiles so the first
            # gather only waits on its own slice): SWDGE descriptor
            # generation never waits behind the big gather transfers
            il_ts, ih_ts = [], []
            g0 = 0
            for nb in GITERS:
                il_t = gip.tile([128, nb * KL * 8], I16, tag=f"il{len(il_ts)}")
                ih_t = gip.tile([128, nb * KH * 8], I16, tag=f"ih{len(ih_ts)}")
                nc.sync.dma_start(out=il_t,
                                  in_=idxlo[:, g0 * KL * 8:(g0 + nb) * KL * 8])
                nc.sync.dma_start(out=ih_t,
                                  in_=idxhi[:, g0 * KH * 8:(g0 + nb) * KH * 8])
                il_ts.append(il_t)
                ih_ts.append(ih_t)
                g0 += nb

            iota_t = cp.tile([128, 128], BF16)
            ident_t = cp.tile([128, 128], BF16)
            w1_t = cp.tile([IN_CH, OUT_CH], BF16)
            w2_t = cp.tile([EDGE_DIM, OUT_CH], BF16)
            b_t = cp.tile([1, OUT_CH], BF16)
            ones_t = cp.tile([1, 128], BF16)
            dummy_t = cp.tile([1, 2], F32)
            pse_hist = []
            nc.sync.dma_start(out=ones_t, in_=ones_h[:, :])
            nc.sync.dma_start(out=iota_t, in_=iota_h[:, :])
            nc.sync.dma_start(out=ident_t, in_=ident_h[:, :])
            nc.sync.dma_start(out=w1_t, in_=W_h[0:IN_CH, :])
            nc.sync.dma_start(out=w2_t, in_=W_h[IN_CH:F, :])
            nc.sync.dma_start(out=b_t, in_=b_h[:, :])

            # software-pipelined edge-stream prefetch, one giter ahead
            gstart = [sum(GITERS[:i]) for i in range(NGI)]
            ea_gs, rr_gs = [None] * NGI, [None] * NGI

            def prefetch_streams(gi):
                nb, s0 = GITERS[gi], gstart[gi]
                ea_g = ep.tile([128, nb * K, EDGE_DIM], FP8, tag="ea", bufs=6)
                rr_g = ep.tile([128, nb * K, 2], F32, tag="rr", bufs=6)
                nc.sync.dma_start(out=ea_g, in_=ea_h[:, s0 * K:(s0 + nb) * K, :])
                nc.sync.dma_start(out=rr_g, in_=rr_h[:, s0 * K:(s0 + nb) * K, :])
                ea_gs[gi], rr_gs[gi] = ea_g, rr_g

            prefetch_streams(0)
            if NGI > 1:
                prefetch_streams(1)

            qn = 0
            for gi, nb in enumerate(GITERS):
                g0 = gstart[gi]
                il_t, ih_t = il_ts[gi], ih_ts[gi]
                nlo, nhi = nb * KL, nb * KH
                xg_lo = gp.tile([128, nlo, IN_CH], BF16, tag="lo")
                xg_hi = gp.tile([128, nhi, IN_CH], BF16, tag="hi")
                ea_g, rr_g = ea_gs[gi], rr_gs[gi]
                if "gather" not in skip:
                    # one lo + one hi call per block so each block's
                    # compute gates only on its own slice of the gather
                    for bb in range(nb):
                        c0, cn = bb * KL, KL
                        nc.gpsimd.dma_gather(
                            xg_lo[:, c0:c0 + cn, :], xlo[:, :],
                            il_t[:, c0 * 8:(c0 + cn) * 8],
                            cn * 128, cn * 128, IN_CH, single_packet=False,
                            queue_num=qn % 4)
                        qn += 1
                        c0, cn = bb * KH, KH
                        nc.gpsimd.dma_gather(
                            xg_hi[:, c0:c0 + cn, :], xhi[:, :],
                            ih_t[:, c0 * 8:(c0 + cn) * 8],
                            cn * 128, cn * 128, IN_CH, single_packet=False,
                            queue_num=qn % 4)
                        qn += 1
                if gi + 2 < NGI:
                    prefetch_streams(gi + 2)

                for bb in range(nb):
                    b = g0 + bb
                    ps_x = pagg.tile([IN_CH, BLK], F32, tag="psx")
                    ps_e = pagg.tile([BLK, EDGE_DIM], F32, tag="pse")
                    if len(pse_hist) >= 2:
                        # one DVE wait that dominates all 32 P-tile
                        # anti-deps of this block (P ring = 2 blocks), so
                        # the per-chunk waits are elided
                        nc.vector.tensor_copy(out=dummy_t[:, 0:1],
                                              in_=pse_hist[-2][0:1, 0:1])
                    pse_hist.append(ps_e)
                    for k in range(K):
                        c = bb * K + k
                        P = pb.tile([128, 128], BF16)
                        if "onehot" not in skip:
                            nc.vector.tensor_scalar(
                                out=P[:],
                                in0=iota_t[:],
                                scalar1=rr_g[:, c, 0:1],
                                scalar2=rr_g[:, c, 1:2],
                                op0=mybir.AluOpType.is_equal,
                                op1=mybir.AluOpType.mult,
                            )
                        if k < KL:
                            rhs_x = xg_lo[:, bb * KL + k, :]
                        else:
                            rhs_x = xg_hi[:, bb * KH + (k - KL), :]
                        if "mm" not in skip:
                            nc.tensor.matmul(ps_x[:], lhsT=rhs_x, rhs=P[:],
                                             start=(k == 0), stop=(k == K - 1))
                            nc.tensor.matmul(ps_e[:], lhsT=P[:], rhs=ea_g[:, c, :],
                                             start=(k == 0), stop=(k == K - 1))

                    aggT_x = st.tile([128, BLK], BF16, tag="aggtx")
                    agg_e = st.tile([BLK, EDGE_DIM], BF16, tag="aggsb")
                    nc.scalar.copy(aggT_x[:], ps_x[:])
                    nc.scalar.copy(agg_e[:], ps_e[:])
                    pt2 = pmisc.tile([EDGE_DIM, BLK], BF16, tag="pt2", bufs=2)
                    nc.tensor.transpose(pt2[:], agg_e[:], ident_t[:])
                    aggT_e = st.tile([EDGE_DIM, BLK], BF16, tag="aggte")
                    nc.scalar.copy(aggT_e[:], pt2[:])
                    ps_o = pmisc.tile([128, OUT_CH], F32, tag="pso", bufs=2)
                    nc.tensor.matmul(ps_o[:], lhsT=aggT_x[:], rhs=w1_t[:],
                                     start=True, stop=False)
                    nc.tensor.matmul(ps_o[:], lhsT=aggT_e[:], rhs=w2_t[:],
                                     start=False, stop=False)
                    # bias via rank-1 matmul: ones[1,128]^T @ b[1,128]
                    nc.tensor.matmul(ps_o[:], lhsT=ones_t[:], rhs=b_t[:],
                                     start=False, stop=True)
                    out_sb = st.tile([128, OUT_CH], BF16, tag="outsb")
                    nc.scalar.copy(out_sb[:], ps_o[:])
                    nc.sync.dma_start(out=out[b * BLK:(b + 1) * BLK, :],
                                      in_=out_sb[:, :])
    nc.finalize()
    return nc


def _wrap16(idx_core):
    """[NB*KX*128] -> [128, NB*KX*8] int16 SWDGE wrapped layout.

    Column j of each 16-partition group holds indices [16j, 16j+16);
    any slice at 16-index granularity is itself well-formed, so one
    flat array serves every per-call slice."""
    n = idx_core.shape[0]
    a = idx_core.reshape(n // 16, 16).T  # [16, n//16]
    return np.ascontiguousarray(np.tile(a, (8, 1)).astype(np.int16))


def _balance_rows(row):
    """Deal rows into NBLOCKS blocks: <=128 rows per block, edge loads as
    even as possible.  Returns block_of_row[N], rrel_of_row[N], max load."""
    deg = np.bincount(row, minlength=N_NODES).astype(np.int64)
    order = np.argsort(-deg, kind="stable")
    loads = np.zeros(NBLOCKS, dtype=np.int64)
    nrows = np.zeros(NBLOCKS, dtype=np.int32)
    block_of_row = np.empty(N_NODES, dtype=np.int32)
    # greedy rounds: biggest remaining rows -> least-loaded blocks.
    # each round hands each block at most one row, so nrows <= 128.
    pos = 0
    while pos < N_NODES:
        nround = min(NBLOCKS, N_NODES - pos)
        rows_r = order[pos:pos + nround]          # degree descending
        border = np.argsort(loads, kind="stable")[:nround]
        block_of_row[rows_r] = border
        loads[border] += deg[rows_r]
        nrows[border] += 1
        pos += nround
    # refinement: move rows off the most-loaded block
    for _ in range(3000):
        bmax = int(np.argmax(loads))
        bmin = int(np.argmin(loads))
        if loads[bmax] - loads[bmin] <= 2 or nrows[bmin] >= 128:
            break
        rows_b = np.flatnonzero(block_of_row == bmax)
        cand = rows_b[deg[rows_b] > 0]
        if cand.size == 0:
            break
        want = (loads[bmax] - loads[bmin]) // 2
        r = cand[int(np.argmin(np.abs(deg[cand] - want)))]
        if deg[r] >= loads[bmax] - loads[bmin]:
            break
        block_of_row[r] = bmin
        loads[bmax] -= deg[r]
        loads[bmin] += deg[r]
        nrows[bmax] -= 1
        nrows[bmin] += 1
    # assign rrel slots within each block
    bsort = np.argsort(block_of_row, kind="stable")
    bo = block_of_row[bsort]
    starts = np.searchsorted(bo, np.arange(NBLOCKS))
    rrel_of_row = np.empty(N_NODES, dtype=np.int32)
    rrel_of_row[bsort] = np.arange(N_NODES) - starts[bo]
    return block_of_row, rrel_of_row, int(loads.max())


def _preprocess(row, col, norm, eattr):
    E = row.shape[0]
    block_of_row, rrel_of_row, maxload = _balance_rows(row)

    KL = KH = 16
    while maxload > (KL + KH) * 128:
        KL += 1
    K = KL + KH

    bid = block_of_row[row]
    rrel = rrel_of_row[row]

    # lo/hi class per edge: 0 = lo-only, 1 = flex, 2 = hi-only
    cls = np.where(col < HI_BASE, 0, np.where(col < SPLIT, 1, 2))

    order = np.argsort(bid * 4 + cls, kind="stable")
    key_s = (bid * 4 + cls)[order]
    grp_start = np.searchsorted(key_s, np.arange(NBLOCKS * 4))
    grp_cnt = np.diff(np.append(grp_start, E)).reshape(NBLOCKS, 4)
    L, Fx, H = grp_cnt[:, 0], grp_cnt[:, 1], grp_cnt[:, 2]
    # flex edges assigned to the lo side: enough that hi fits, at most
    # lo capacity
    take = np.clip(H + Fx - KH * 128, 0, np.minimum(Fx, KL * 128 - L))
    if np.any(L + take > KL * 128) or np.any(H + Fx - take > KH * 128):
        KL += 1
        KH += 1
        K = KL + KH
        take = np.clip(H + Fx - KH * 128, 0, np.minimum(Fx, KL * 128 - L))

    pos_in_grp = np.arange(E) - grp_start[key_s]
    cls_s = key_s & 3
    bid_s = key_s >> 2
    is_lo = (cls_s == 0) | ((cls_s == 1) & (pos_in_grp < take[bid_s]))
    slot_lo = np.where(cls_s == 0, pos_in_grp, L[bid_s] + pos_in_grp)
    slot_hi = np.where(cls_s == 1, pos_in_grp - take[bid_s],
                       (Fx - take)[bid_s] + pos_in_grp)
    slot = np.where(is_lo, slot_lo, KL * 128 + slot_hi)

    core_s = bid_s // NB
    bloc_s = bid_s % NB
    dst = (core_s * NB + bloc_s) * (K * 128) + slot

    S = N_CORES * NB * K * 128
    col_pad = np.zeros(S, dtype=np.int32)
    col_pad.reshape(-1, K * 128)[:, KL * 128:] = HI_BASE
    norm_pad = np.zeros(S, dtype=np.float32)
    rrel_pad = np.zeros(S, dtype=np.float32)
    col_pad[dst] = col[order]
    norm_pad[dst] = norm[order]
    rrel_pad[dst] = rrel[order]
    ea_pad = np.zeros((S, EDGE_DIM), dtype=ml_dtypes.float8_e4m3)
    ea_pad[dst] = eattr[order].astype(ml_dtypes.float8_e4m3)

    # gather index streams, wrapped-16, whole core
    colr = col_pad.reshape(N_CORES, NB, K, 128)
    lo_flat = np.ascontiguousarray(colr[:, :, :KL, :]).reshape(N_CORES, -1)
    hi_flat = np.ascontiguousarray(colr[:, :, KL:, :] - HI_BASE).reshape(N_CORES, -1)
    idxlo = [_wrap16(lo_flat[c]) for c in range(N_CORES)]
    idxhi = [_wrap16(hi_flat[c]) for c in range(N_CORES)]

    # partition-major streams: edge (block b, chunk k, partition p)
    # -> [core, p, b*K+k, :]
    ea4 = ea_pad.reshape(N_CORES, NB * K, 128, EDGE_DIM)
    ea_h = np.ascontiguousarray(ea4.transpose(0, 2, 1, 3))
    rr2 = np.stack([rrel_pad, norm_pad], axis=1)
    rr4 = rr2.reshape(N_CORES, NB * K, 128, 2)
    rr_h = np.ascontiguousarray(rr4.transpose(0, 2, 1, 3))
    return KL, KH, idxlo, idxhi, ea_h, rr_h, block_of_row, rrel_of_row


def _run_device(x, row, col, norm, eattr, W, b):
    from concourse import bass_utils

    (KL, KH, idxlo, idxhi, ea_h, rr_h,
     block_of_row, rrel_of_row) = _preprocess(row, col, norm, eattr)
    key = (KL, KH)
    if key not in _NC_CACHE:
        _NC_CACHE.clear()
        _NC_CACHE[key] = _build_nc(KL, KH)
    nc = _NC_CACHE[key]

    x_bf = _to_bf16(x)
    xlo = np.ascontiguousarray(x_bf[:SPLIT])
    xhi = np.ascontiguousarray(x_bf[HI_BASE:])
    iota_h = np.tile(
        np.arange(128, dtype=np.float32).astype(ml_dtypes.bfloat16)[None, :],
        (128, 1))
    ident_h = np.eye(128, dtype=np.float32).astype(ml_dtypes.bfloat16)
    W_bf = _to_bf16(W)
    b_h = _to_bf16(np.asarray(b, dtype=np.float32))[None, :]
    ones_h = np.ones((1, 128), dtype=np.float32).astype(ml_dtypes.bfloat16)

    in_maps = []
    for c in range(N_CORES):
        in_maps.append({
            "xlo": xlo, "xhi": xhi,
            "idxlo": idxlo[c], "idxhi": idxhi[c],
            "ea_h": ea_h[c], "rr_h": rr_h[c],
            "iota_h": iota_h, "ident_h": ident_h,
            "W_h": W_bf, "b_h": b_h, "ones_h": ones_h,
        })
    res = bass_utils.run_bass_kernel_spmd(nc, in_maps,
                                          core_ids=list(range(N_CORES)))
    allout = np.stack([np.asarray(res.results[i]["out"], dtype=np.float32)
                       for i in range(N_CORES)], axis=0)
    # un-permute: row r lives at (core, block*128 + rrel)
    core_of_row = block_of_row // NB
    slot_of_row = (block_of_row % NB) * BLK + rrel_of_row
    return np.ascontiguousarray(allout[core_of_row, slot_of_row])


def _segment_sum(msg, row, n):
    order = np.argsort(row, kind="stable")
    rs = row[order]
    ms = msg[order]
    starts = np.concatenate(([0], np.flatnonzero(np.diff(rs)) + 1))
    sums = np.add.reduceat(ms, starts, axis=0)
    out = np.zeros((n, msg.shape[1]), dtype=msg.dtype)
    out[rs[starts]] = sums
    return out


def _cpu_fallback(x, row, col, norm, eattr, W, b):
    msg = np.empty((N_EDGES, F), dtype=np.float32)
    np.multiply(x[col], norm[:, None], out=msg[:, :IN_CH])
    np.multiply(eattr, norm[:, None], out=msg[:, IN_CH:])
    agg = _segment_sum(msg, row, N_NODES)
    return (agg @ W + b[None, :]).astype(np.float32)


def kernel(**inputs) -> np.ndarray:
    x = np.ascontiguousarray(inputs["x"], dtype=np.float32)
    row = np.ascontiguousarray(inputs["row"]).astype(np.int64)
    col = np.ascontiguousarray(inputs["col"]).astype(np.int64)
    norm = np.ascontiguousarray(inputs["norm"], dtype=np.float32)
    eattr = np.ascontiguousarray(inputs["edge_attr"], dtype=np.float32)
    W = np.ascontiguousarray(inputs["W"], dtype=np.float32)
    b = np.ascontiguousarray(inputs["b"], dtype=np.float32)
    try:
        return _run_device(x, row, col, norm, eattr, W, b)
    except Exception:
        import traceback
        traceback.print_exc()
        return _cpu_fallback(x, row, col, norm, eattr, W, b)


# revision 22
# speedup vs baseline: 1.0441x; 1.0441x over previous
"""GCNConv message-passing kernel for 8 Trainium2 NeuronCores.

Strategy (edge/graph parallelism, sharded by destination row):
  - 50000 rows are dealt into 392 blocks (8 cores x 49) by a degree
    balancer so every block holds <= 128 rows and <= 4096 edges; each
    block is exactly K=32 chunks of 128 edges (KL=16 lo + KH=16 hi).
  - the gather table is split into two OVERLAPPING halves
    xlo=x[0:32768], xhi=x[17232:50000] so cols in [17232,32768) can be
    assigned to either side ("flex"), letting every block fill exactly
    KL lo-chunks and KH hi-chunks with no split padding.
  - on device, per chunk: SWDGE dma_gather fetches x[col] rows (bf16),
    DVE builds a norm-scaled one-hot P[e,r] = norm_e * (row_rel_e == r),
    PE accumulates psum_x[r,:] += P^T @ x_g and psum_e[r,:] += P^T @ ea
  - per block: transpose agg, apply W (bf16), add bias, DMA out
  - edge_attr streamed as fp8e4m3, output stored bf16 (CPU upcasts)
  - all gather index tiles are DMAed up front so SWDGE descriptor
    generation never waits behind the big gather transfers; one lo and
    one hi dma_gather call per block so a block's compute gates only on
    its own slice of the gather stream
  - x-side scatter emits agg^T directly (lhsT=x_g, rhs=P), ea-side uses
    the cheap orientation (out free dim 32) plus one PE transpose
  - a per-block dummy DVE read of the 2-blocks-ago PSUM tile emits one
    dominating cross-engine wait so the per-chunk P anti-dep waits are
    elided (DVE SEQ cadence 140ns -> 70ns per chunk)
  - giter sizes (4,...,4,2,2,2,1,1,1): small tail giters so little
    compute trails the final gather DMA
  - no collectives needed (cores own disjoint output rows)
"""
import sys
import numpy as np
import ml_dtypes

for _p in ("/opt/trn_rl_repo", "/root/.axon_site/_ro/trn_rl_repo"):
    if _p not in sys.path:
        sys.path.insert(0, _p)

N_NODES = 50000
N_EDGES = 1600000
IN_CH = 128
EDGE_DIM = 32
OUT_CH = 128
F = IN_CH + EDGE_DIM            # 160
N_CORES = 8
BLK = 128
NB = 49                         # blocks per core
NBLOCKS = N_CORES * NB          # 392
SLOTS = NB * BLK                # 6272 output slots per core
SPLIT = 32768                   # int16 gather index limit
HI_BASE = N_NODES - SPLIT       # 17232: xhi = x[HI_BASE:]
GITERS = (4,) * 10 + (2, 2, 2, 1, 1, 1)  # blocks per giter (sum = 49); small tail
CPC = 64                        # chunks per dma_gather call (8192-idx cap)

_NC_CACHE = {}


def _to_bf16(a):
    """fast f32 -> bf16 with round-to-nearest-ish."""
    u = np.ascontiguousarray(a, dtype=np.float32).view(np.uint32)
    return ((u + 0x8000) >> 16).astype(np.uint16).view(ml_dtypes.bfloat16)


def _build_nc(KL, KH, skip=()):
    from concourse import bacc, mybir
    from concourse.tile import TileContext

    K = KL + KH
    BF16 = mybir.dt.bfloat16
    F32 = mybir.dt.float32
    FP8 = mybir.dt.float8e4
    I16 = mybir.dt.int16

    nc = bacc.Bacc(None, target_bir_lowering=False, num_swdge_queues=4)
    xlo = nc.dram_tensor("xlo", [SPLIT, IN_CH], BF16, kind="ExternalInput")
    xhi = nc.dram_tensor("xhi", [N_NODES - HI_BASE, IN_CH], BF16,
                         kind="ExternalInput")
    idxlo = nc.dram_tensor("idxlo", [128, NB * KL * 8], I16, kind="ExternalInput")
    idxhi = nc.dram_tensor("idxhi", [128, NB * KH * 8], I16, kind="ExternalInput")
    NGI = len(GITERS)
    # partition-major streams over the whole core: [p, b*K + k, :]
    ea_h = nc.dram_tensor("ea_h", [128, NB * K, EDGE_DIM], FP8,
                          kind="ExternalInput")
    rr_h = nc.dram_tensor("rr_h", [128, NB * K, 2], F32, kind="ExternalInput")
    iota_h = nc.dram_tensor("iota_h", [128, 128], BF16, kind="ExternalInput")
    ident_h = nc.dram_tensor("ident_h", [128, 128], BF16, kind="ExternalInput")
    W_h = nc.dram_tensor("W_h", [F, OUT_CH], BF16, kind="ExternalInput")
    b_h = nc.dram_tensor("b_h", [1, OUT_CH], BF16, kind="ExternalInput")
    ones_h = nc.dram_tensor("ones_h", [1, 128], BF16, kind="ExternalInput")
    out = nc.dram_tensor("out", [128, NB, OUT_CH], BF16, kind="ExternalOutput")

    with TileContext(nc) as tc:
        with tc.tile_pool(name="const", bufs=1) as cp, \
             tc.tile_pool(name="gidx", bufs=1) as gip, \
             tc.tile_pool(name="gbuf", bufs=3) as gp, \
             tc.tile_pool(name="ebuf", bufs=3) as ep, \
             tc.tile_pool(name="pbuf", bufs=64) as pb, \
             tc.tile_pool(name="stage", bufs=2) as st, \
             tc.tile_pool(name="agg", bufs=2, space="PSUM") as pagg, \
             tc.tile_pool(name="misc", bufs=1, space="PSUM") as pmisc:
            # gather index tiles FIRST (per-giter tiles so the first
            # gather only waits on its own slice): SWDGE descriptor
            # generation never waits behind the big gather transfers
            il_ts, ih_ts = [], []
            g0 = 0
            for nb in GITERS:
                il_t = gip.tile([128, nb * KL * 8], I16, tag=f"il{len(il_ts)}")
                ih_t = gip.tile([128, nb * KH * 8], I16, tag=f"ih{len(ih_ts)}")
                nc.sync.dma_start(out=il_t,
                                  in_=idxlo[:, g0 * KL * 8:(g0 + nb) * KL * 8])
                nc.sync.dma_start(out=ih_t,
                                  in_=idxhi[:, g0 * KH * 8:(g0 + nb) * KH * 8])
                il_ts.append(il_t)
                ih_ts.append(ih_t)
                g0 += nb

            iota_t = cp.tile([128, 128], BF16)
            ident_t = cp.tile([128, 128], BF16)
            w1_t = cp.tile([IN_CH, OUT_CH], BF16)
            w2_t = cp.tile([EDGE_DIM, OUT_CH], BF16)
            b_t = cp.tile([1, OUT_CH], BF16)
            ones_t = cp.tile([1, 128], BF16)
            dummy_t = cp.tile([1, 2], F32)
            pse_hist = []
            nc.sync.dma_start(out=ones_t, in_=ones_h[:, :])
            nc.sync.dma_start(out=iota_t, in_=iota_h[:, :])
            nc.sync.dma_start(out=ident_t, in_=ident_h[:, :])
            nc.sync.dma_start(out=w1_t, in_=W_h[0:IN_CH, :])
            nc.sync.dma_start(out=w2_t, in_=W_h[IN_CH:F, :])
            nc.sync.dma_start(out=b_t, in_=b_h[:, :])

            # software-pipelined edge-stream prefetch, one giter ahead
            gstart = [sum(GITERS[:i]) for i in range(NGI)]
            ea_gs, rr_gs = [None] * NGI, [None] * NGI

            def prefetch_streams(gi):
                nb, s0 = GITERS[gi], gstart[gi]
                ea_g = ep.tile([128, nb * K, EDGE_DIM], FP8, tag="ea", bufs=6)
                rr_g = ep.tile([128, nb * K, 2], F32, tag="rr", bufs=6)
                nc.sync.dma_start(out=ea_g, in_=ea_h[:, s0 * K:(s0 + nb) * K, :])
                nc.sync.dma_start(out=rr_g, in_=rr_h[:, s0 * K:(s0 + nb) * K, :])
                ea_gs[gi], rr_gs[gi] = ea_g, rr_g

            prefetch_streams(0)
            if NGI > 1:
                prefetch_streams(1)
            out_pair = [None]

            qn = 0
            for gi, nb in enumerate(GITERS):
                g0 = gstart[gi]
                il_t, ih_t = il_ts[gi], ih_ts[gi]
                nlo, nhi = nb * KL, nb * KH
                xg_lo = gp.tile([128, nlo, IN_CH], BF16, tag="lo")
                xg_hi = gp.tile([128, nhi, IN_CH], BF16, tag="hi")
                ea_g, rr_g = ea_gs[gi], rr_gs[gi]
                if "gather" not in skip:
                    # one lo + one hi call per block so each block's
                    # compute gates only on its own slice of the gather
                    for bb in range(nb):
                        c0, cn = bb * KL, KL
                        nc.gpsimd.dma_gather(
                            xg_lo[:, c0:c0 + cn, :], xlo[:, :],
                            il_t[:, c0 * 8:(c0 + cn) * 8],
                            cn * 128, cn * 128, IN_CH, single_packet=False,
                            queue_num=qn % 4)
                        qn += 1
                        c0, cn = bb * KH, KH
                        nc.gpsimd.dma_gather(
                            xg_hi[:, c0:c0 + cn, :], xhi[:, :],
                            ih_t[:, c0 * 8:(c0 + cn) * 8],
                            cn * 128, cn * 128, IN_CH, single_packet=False,
                            queue_num=qn % 4)
                        qn += 1
                if gi + 2 < NGI:
                    prefetch_streams(gi + 2)

                for bb in range(nb):
                    b = g0 + bb
                    ps_x = pagg.tile([IN_CH, BLK], F32, tag="psx")
                    ps_e = pagg.tile([BLK, EDGE_DIM], F32, tag="pse")
                    if len(pse_hist) >= 2:
                        # one DVE wait that dominates all 32 P-tile
                        # anti-deps of this block (P ring = 2 blocks), so
                        # the per-chunk waits are elided
                        nc.vector.tensor_copy(out=dummy_t[:, 0:1],
                                              in_=pse_hist[-2][0:1, 0:1])
                    pse_hist.append(ps_e)
                    for k in range(K):
                        c = bb * K + k
                        P = pb.tile([128, 128], BF16)
                        if "onehot" not in skip:
                            nc.vector.tensor_scalar(
                                out=P[:],
                                in0=iota_t[:],
                                scalar1=rr_g[:, c, 0:1],
                                scalar2=rr_g[:, c, 1:2],
                                op0=mybir.AluOpType.is_equal,
                                op1=mybir.AluOpType.mult,
                            )
                        if k < KL:
                            rhs_x = xg_lo[:, bb * KL + k, :]
                        else:
                            rhs_x = xg_hi[:, bb * KH + (k - KL), :]
                        if "mm" not in skip:
                            nc.tensor.matmul(ps_x[:], lhsT=rhs_x, rhs=P[:],
                                             start=(k == 0), stop=(k == K - 1))
                            nc.tensor.matmul(ps_e[:], lhsT=P[:], rhs=ea_g[:, c, :],
                                             start=(k == 0), stop=(k == K - 1))

                    aggT_x = st.tile([128, BLK], BF16, tag="aggtx")
                    agg_e = st.tile([BLK, EDGE_DIM], BF16, tag="aggsb")
                    nc.scalar.copy(aggT_x[:], ps_x[:])
                    nc.scalar.copy(agg_e[:], ps_e[:])
                    pt2 = pmisc.tile([EDGE_DIM, BLK], BF16, tag="pt2", bufs=2)
                    nc.tensor.transpose(pt2[:], agg_e[:], ident_t[:])
                    aggT_e = st.tile([EDGE_DIM, BLK], BF16, tag="aggte")
                    nc.scalar.copy(aggT_e[:], pt2[:])
                    ps_o = pmisc.tile([128, OUT_CH], F32, tag="pso", bufs=2)
                    nc.tensor.matmul(ps_o[:], lhsT=aggT_x[:], rhs=w1_t[:],
                                     start=True, stop=False)
                    nc.tensor.matmul(ps_o[:], lhsT=aggT_e[:], rhs=w2_t[:],
                                     start=False, stop=False)
                    # bias via rank-1 matmul: ones[1,128]^T @ b[1,128]
                    nc.tensor.matmul(ps_o[:], lhsT=ones_t[:], rhs=b_t[:],
                                     start=False, stop=True)
                    if b % 2 == 0:
                        out_pair[0] = st.tile([128, 2, OUT_CH], BF16, tag="outsb", name="out_pair")
                    nc.scalar.copy(out_pair[0][:, b % 2, :], ps_o[:])
                    if b % 2 == 1:
                        nc.sync.dma_start(out=out[:, b - 1:b + 1, :],
                                          in_=out_pair[0][:, :, :])
                    elif b == NB - 1:
                        nc.sync.dma_start(out=out[:, b:b + 1, :],
                                          in_=out_pair[0][:, 0:1, :])
    nc.finalize()
    return nc


def _wrap16(idx_core):
    """[NB*KX*128] -> [128, NB*KX*8] int16 SWDGE wrapped layout.

    Column j of each 16-partition group holds indices [16j, 16j+16);
    any slice at 16-index granularity is itself well-formed, so one
    flat array serves every per-call slice."""
    n = idx_core.shape[0]
    a = idx_core.reshape(n // 16, 16).T  # [16, n//16]
    return np.ascontiguousarray(np.tile(a, (8, 1)).astype(np.int16))


def _balance_rows(row):
    """Deal rows into NBLOCKS blocks: <=128 rows per block, edge loads as
    even as possible.  Returns block_of_row[N], rrel_of_row[N], max load."""
    deg = np.bincount(row, minlength=N_NODES).astype(np.int64)
    order = np.argsort(-deg, kind="stable")
    loads = np.zeros(NBLOCKS, dtype=np.int64)
    nrows = np.zeros(NBLOCKS, dtype=np.int32)
    block_of_row = np.empty(N_NODES, dtype=np.int32)
    # greedy rounds: biggest remaining rows -> least-loaded blocks.
    # each round hands each block at most one row, so nrows <= 128.
    pos = 0
    while pos < N_NODES:
        nround = min(NBLOCKS, N_NODES - pos)
        rows_r = order[pos:pos + nround]          # degree descending
        border = np.argsort(loads, kind="stable")[:nround]
        block_of_row[rows_r] = border
        loads[border] += deg[rows_r]
        nrows[border] += 1
        pos += nround
    # refinement: move rows off the most-loaded block
    for _ in range(3000):
        bmax = int(np.argmax(loads))
        bmin = int(np.argmin(loads))
        if loads[bmax] - loads[bmin] <= 2 or nrows[bmin] >= 128:
            break
        rows_b = np.flatnonzero(block_of_row == bmax)
        cand = rows_b[deg[rows_b] > 0]
        if cand.size == 0:
            break
        want = (loads[bmax] - loads[bmin]) // 2
        r = cand[int(np.argmin(np.abs(deg[cand] - want)))]
        if deg[r] >= loads[bmax] - loads[bmin]:
            break
        block_of_row[r] = bmin
        loads[bmax] -= deg[r]
        loads[bmin] += deg[r]
        nrows[bmax] -= 1
        nrows[bmin] += 1
    # assign rrel slots within each block
    bsort = np.argsort(block_of_row, kind="stable")
    bo = block_of_row[bsort]
    starts = np.searchsorted(bo, np.arange(NBLOCKS))
    rrel_of_row = np.empty(N_NODES, dtype=np.int32)
    rrel_of_row[bsort] = np.arange(N_NODES) - starts[bo]
    return block_of_row, rrel_of_row, int(loads.max())


def _preprocess(row, col, norm, eattr):
    E = row.shape[0]
    block_of_row, rrel_of_row, maxload = _balance_rows(row)

    KL = KH = 16
    while maxload > (KL + KH) * 128:
        KL += 1
    K = KL + KH

    bid = block_of_row[row]
    rrel = rrel_of_row[row]

    # lo/hi class per edge: 0 = lo-only, 1 = flex, 2 = hi-only
    cls = np.where(col < HI_BASE, 0, np.where(col < SPLIT, 1, 2))

    order = np.argsort(bid * 4 + cls, kind="stable")
    key_s = (bid * 4 + cls)[order]
    grp_start = np.searchsorted(key_s, np.arange(NBLOCKS * 4))
    grp_cnt = np.diff(np.append(grp_start, E)).reshape(NBLOCKS, 4)
    L, Fx, H = grp_cnt[:, 0], grp_cnt[:, 1], grp_cnt[:, 2]
    # flex edges assigned to the lo side: enough that hi fits, at most
    # lo capacity
    take = np.clip(H + Fx - KH * 128, 0, np.minimum(Fx, KL * 128 - L))
    if np.any(L + take > KL * 128) or np.any(H + Fx - take > KH * 128):
        KL += 1
        KH += 1
        K = KL + KH
        take = np.clip(H + Fx - KH * 128, 0, np.minimum(Fx, KL * 128 - L))

    pos_in_grp = np.arange(E) - grp_start[key_s]
    cls_s = key_s & 3
    bid_s = key_s >> 2
    is_lo = (cls_s == 0) | ((cls_s == 1) & (pos_in_grp < take[bid_s]))
    slot_lo = np.where(cls_s == 0, pos_in_grp, L[bid_s] + pos_in_grp)
    slot_hi = np.where(cls_s == 1, pos_in_grp - take[bid_s],
                       (Fx - take)[bid_s] + pos_in_grp)
    slot = np.where(is_lo, slot_lo, KL * 128 + slot_hi)

    core_s = bid_s // NB
    bloc_s = bid_s % NB
    dst = (core_s * NB + bloc_s) * (K * 128) + slot

    S = N_CORES * NB * K * 128
    col_pad = np.zeros(S, dtype=np.int32)
    col_pad.reshape(-1, K * 128)[:, KL * 128:] = HI_BASE
    norm_pad = np.zeros(S, dtype=np.float32)
    rrel_pad = np.zeros(S, dtype=np.float32)
    col_pad[dst] = col[order]
    norm_pad[dst] = norm[order]
    rrel_pad[dst] = rrel[order]
    ea_pad = np.zeros((S, EDGE_DIM), dtype=ml_dtypes.float8_e4m3)
    ea_pad[dst] = eattr[order].astype(ml_dtypes.float8_e4m3)

    # gather index streams, wrapped-16, whole core
    colr = col_pad.reshape(N_CORES, NB, K, 128)
    lo_flat = np.ascontiguousarray(colr[:, :, :KL, :]).reshape(N_CORES, -1)
    hi_flat = np.ascontiguousarray(colr[:, :, KL:, :] - HI_BASE).reshape(N_CORES, -1)
    idxlo = [_wrap16(lo_flat[c]) for c in range(N_CORES)]
    idxhi = [_wrap16(hi_flat[c]) for c in range(N_CORES)]

    # partition-major streams: edge (block b, chunk k, partition p)
    # -> [core, p, b*K+k, :]
    ea4 = ea_pad.reshape(N_CORES, NB * K, 128, EDGE_DIM)
    ea_h = np.ascontiguousarray(ea4.transpose(0, 2, 1, 3))
    rr2 = np.stack([rrel_pad, norm_pad], axis=1)
    rr4 = rr2.reshape(N_CORES, NB * K, 128, 2)
    rr_h = np.ascontiguousarray(rr4.transpose(0, 2, 1, 3))
    return KL, KH, idxlo, idxhi, ea_h, rr_h, block_of_row, rrel_of_row


def _run_device(x, row, col, norm, eattr, W, b):
    from concourse import bass_utils

    (KL, KH, idxlo, idxhi, ea_h, rr_h,
     block_of_row, rrel_of_row) = _preprocess(row, col, norm, eattr)
    key = (KL, KH)
    if key not in _NC_CACHE:
        _NC_CACHE.clear()
        _NC_CACHE[key] = _build_nc(KL, KH)
    nc = _NC_CACHE[key]

    x_bf = _to_bf16(x)
    xlo = np.ascontiguousarray(x_bf[:SPLIT])
    xhi = np.ascontiguousarray(x_bf[HI_BASE:])
    iota_h = np.tile(
        np.arange(128, dtype=np.float32).astype(ml_dtypes.bfloat16)[None, :],
        (128, 1))
    ident_h = np.eye(128, dtype=np.float32).astype(ml_dtypes.bfloat16)
    W_bf = _to_bf16(W)
    b_h = _to_bf16(np.asarray(b, dtype=np.float32))[None, :]
    ones_h = np.ones((1, 128), dtype=np.float32).astype(ml_dtypes.bfloat16)

    in_maps = []
    for c in range(N_CORES):
        in_maps.append({
            "xlo": xlo, "xhi": xhi,
            "idxlo": idxlo[c], "idxhi": idxhi[c],
            "ea_h": ea_h[c], "rr_h": rr_h[c],
            "iota_h": iota_h, "ident_h": ident_h,
            "W_h": W_bf, "b_h": b_h, "ones_h": ones_h,
        })
    res = bass_utils.run_bass_kernel_spmd(nc, in_maps,
                                          core_ids=list(range(N_CORES)))
    allout = np.stack([np.asarray(res.results[i]["out"], dtype=np.float32)
                       for i in range(N_CORES)], axis=0)  # [8, 128, NB, 128]
    # un-permute: row r lives at (core, partition rrel, local block)
    core_of_row = block_of_row // NB
    bloc_of_row = block_of_row % NB
    return np.ascontiguousarray(
        allout[core_of_row, rrel_of_row, bloc_of_row])


def _segment_sum(msg, row, n):
    order = np.argsort(row, kind="stable")
    rs = row[order]
    ms = msg[order]
    starts = np.concatenate(([0], np.flatnonzero(np.diff(rs)) + 1))
    sums = np.add.reduceat(ms, starts, axis=0)
    out = np.zeros((n, msg.shape[1]), dtype=msg.dtype)
    out[rs[starts]] = sums
    return out


def _cpu_fallback(x, row, col, norm, eattr, W, b):
    msg = np.empty((N_EDGES, F), dtype=np.float32)
    np.multiply(x[col], norm[:, None], out=msg[:, :IN_CH])
    np.multiply(eattr, norm[:, None], out=msg[:, IN_CH:])
    agg = _segment_sum(msg, row, N_NODES)
    return (agg @ W + b[None, :]).astype(np.float32)


def kernel(**inputs) -> np.ndarray:
    x = np.ascontiguousarray(inputs["x"], dtype=np.float32)
    row = np.ascontiguousarray(inputs["row"]).astype(np.int64)
    col = np.ascontiguousarray(inputs["col"]).astype(np.int64)
    norm = np.ascontiguousarray(inputs["norm"], dtype=np.float32)
    eattr = np.ascontiguousarray(inputs["edge_attr"], dtype=np.float32)
    W = np.ascontiguousarray(inputs["W"], dtype=np.float32)
    b = np.ascontiguousarray(inputs["b"], dtype=np.float32)
    try:
        return _run_device(x, row, col, norm, eattr, W, b)
    except Exception:
        import traceback
        traceback.print_exc()
        return _cpu_fallback(x, row, col, norm, eattr, W, b)


# revision 25
# speedup vs baseline: 1.0526x; 1.0082x over previous
"""GCNConv message-passing kernel for 8 Trainium2 NeuronCores.

Strategy (edge/graph parallelism, sharded by destination row):
  - 50000 rows are dealt into 392 blocks (8 cores x 49) by a degree
    balancer so every block holds <= 128 rows and <= 4096 edges; each
    block is exactly K=32 chunks of 128 edges (KL=16 lo + KH=16 hi).
  - the gather table is split into two OVERLAPPING halves
    xlo=x[0:32768], xhi=x[17232:50000] so cols in [17232,32768) can be
    assigned to either side ("flex"), letting every block fill exactly
    KL lo-chunks and KH hi-chunks with no split padding.
  - on device, per chunk: SWDGE dma_gather fetches x[col] rows (bf16),
    DVE builds a norm-scaled one-hot P[e,r] = norm_e * (row_rel_e == r),
    PE accumulates psum_x[r,:] += P^T @ x_g and psum_e[r,:] += P^T @ ea
  - per block: transpose agg, apply W (bf16), add bias, DMA out
  - edge_attr streamed as fp8e4m3, output stored bf16 (CPU upcasts)
  - all gather index tiles are DMAed up front so SWDGE descriptor
    generation never waits behind the big gather transfers; one lo and
    one hi dma_gather call per block so a block's compute gates only on
    its own slice of the gather stream
  - x-side scatter emits agg^T directly (lhsT=x_g, rhs=P), ea-side uses
    the cheap orientation (out free dim 32) plus one PE transpose
  - a per-block dummy DVE read of the 2-blocks-ago PSUM tile emits one
    dominating cross-engine wait so the per-chunk P anti-dep waits are
    elided (DVE SEQ cadence 140ns -> 70ns per chunk)
  - giter sizes (4,...,4,2,2,2,1,1,1): small tail giters so little
    compute trails the final gather DMA
  - no collectives needed (cores own disjoint output rows)
"""
import sys
import numpy as np
import ml_dtypes

for _p in ("/opt/trn_rl_repo", "/root/.axon_site/_ro/trn_rl_repo"):
    if _p not in sys.path:
        sys.path.insert(0, _p)

N_NODES = 50000
N_EDGES = 1600000
IN_CH = 128
EDGE_DIM = 32
OUT_CH = 128
F = IN_CH + EDGE_DIM            # 160
N_CORES = 8
BLK = 128
NB = 49                         # blocks per core
NBLOCKS = N_CORES * NB          # 392
SLOTS = NB * BLK                # 6272 output slots per core
SPLIT = 32768                   # int16 gather index limit
HI_BASE = N_NODES - SPLIT       # 17232: xhi = x[HI_BASE:]
GITERS = (4,) * 10 + (2, 2, 2, 1, 1, 1)  # blocks per giter (sum = 49); small tail
CPC = 64                        # chunks per dma_gather call (8192-idx cap)

_NC_CACHE = {}


def _to_bf16(a):
    """fast f32 -> bf16 with round-to-nearest-ish."""
    u = np.ascontiguousarray(a, dtype=np.float32).view(np.uint32)
    return ((u + 0x8000) >> 16).astype(np.uint16).view(ml_dtypes.bfloat16)


def _build_nc(KL, KH, skip=()):
    from concourse import bacc, mybir
    from concourse.tile import TileContext

    K = KL + KH
    BF16 = mybir.dt.bfloat16
    F32 = mybir.dt.float32
    FP8 = mybir.dt.float8e4
    I16 = mybir.dt.int16

    nc = bacc.Bacc(None, target_bir_lowering=False, num_swdge_queues=4)
    xlo = nc.dram_tensor("xlo", [SPLIT, IN_CH], BF16, kind="ExternalInput")
    xhi = nc.dram_tensor("xhi", [N_NODES - HI_BASE, IN_CH], BF16,
                         kind="ExternalInput")
    idxlo = nc.dram_tensor("idxlo", [128, NB * KL * 8], I16, kind="ExternalInput")
    idxhi = nc.dram_tensor("idxhi", [128, NB * KH * 8], I16, kind="ExternalInput")
    NGI = len(GITERS)
    # partition-major streams over the whole core: [p, b*K + k, :]
    ea_h = nc.dram_tensor("ea_h", [128, NB * K, EDGE_DIM], FP8,
                          kind="ExternalInput")
    rr_h = nc.dram_tensor("rr_h", [128, NB * K, 2], F32, kind="ExternalInput")
    iota_h = nc.dram_tensor("iota_h", [128, 128], BF16, kind="ExternalInput")
    ident_h = nc.dram_tensor("ident_h", [128, 128], BF16, kind="ExternalInput")
    W_h = nc.dram_tensor("W_h", [F, OUT_CH], BF16, kind="ExternalInput")
    b_h = nc.dram_tensor("b_h", [1, OUT_CH], BF16, kind="ExternalInput")
    ones_h = nc.dram_tensor("ones_h", [1, 128], BF16, kind="ExternalInput")
    out = nc.dram_tensor("out", [128, NB, OUT_CH], BF16, kind="ExternalOutput")

    with TileContext(nc) as tc:
        with tc.tile_pool(name="const", bufs=1) as cp, \
             tc.tile_pool(name="gidx", bufs=1) as gip, \
             tc.tile_pool(name="gbuf", bufs=3) as gp, \
             tc.tile_pool(name="ebuf", bufs=3) as ep, \
             tc.tile_pool(name="pbuf", bufs=64) as pb, \
             tc.tile_pool(name="stage", bufs=2) as st, \
             tc.tile_pool(name="agg", bufs=2, space="PSUM") as pagg, \
             tc.tile_pool(name="misc", bufs=1, space="PSUM") as pmisc:
            # gather index tiles FIRST (per-giter tiles so the first
            # gather only waits on its own slice): SWDGE descriptor
            # generation never waits behind the big gather transfers
            il_ts, ih_ts = [], []
            g0 = 0
            for nb in GITERS:
                il_t = gip.tile([128, nb * KL * 8], I16, tag=f"il{len(il_ts)}")
                ih_t = gip.tile([128, nb * KH * 8], I16, tag=f"ih{len(ih_ts)}")
                nc.sync.dma_start(out=il_t,
                                  in_=idxlo[:, g0 * KL * 8:(g0 + nb) * KL * 8])
                nc.sync.dma_start(out=ih_t,
                                  in_=idxhi[:, g0 * KH * 8:(g0 + nb) * KH * 8])
                il_ts.append(il_t)
                ih_ts.append(ih_t)
                g0 += nb

            iota_t = cp.tile([128, 128], BF16)
            ident_t = cp.tile([128, 128], BF16)
            w1_t = cp.tile([IN_CH, OUT_CH], BF16)
            w2_t = cp.tile([EDGE_DIM, OUT_CH], BF16)
            b_t = cp.tile([1, OUT_CH], BF16)
            ones_t = cp.tile([1, 128], BF16)
            dummy_t = cp.tile([1, 2], F32)
            pse_hist = []
            nc.sync.dma_start(out=ones_t, in_=ones_h[:, :])
            nc.sync.dma_start(out=iota_t, in_=iota_h[:, :])
            nc.sync.dma_start(out=ident_t, in_=ident_h[:, :])
            nc.sync.dma_start(out=w1_t, in_=W_h[0:IN_CH, :])
            nc.sync.dma_start(out=w2_t, in_=W_h[IN_CH:F, :])
            nc.sync.dma_start(out=b_t, in_=b_h[:, :])

            # software-pipelined edge-stream prefetch, one giter ahead
            gstart = [sum(GITERS[:i]) for i in range(NGI)]
            ea_gs, rr_gs = [None] * NGI, [None] * NGI

            def prefetch_streams(gi):
                nb, s0 = GITERS[gi], gstart[gi]
                ea_g = ep.tile([128, nb * K, EDGE_DIM], FP8, tag="ea", bufs=8)
                rr_g = ep.tile([128, nb * K, 2], F32, tag="rr", bufs=8)
                nc.sync.dma_start(out=ea_g, in_=ea_h[:, s0 * K:(s0 + nb) * K, :])
                nc.sync.dma_start(out=rr_g, in_=rr_h[:, s0 * K:(s0 + nb) * K, :])
                ea_gs[gi], rr_gs[gi] = ea_g, rr_g

            prefetch_streams(0)
            if NGI > 1:
                prefetch_streams(1)
            out_pair = [None]

            qn = 0
            for gi, nb in enumerate(GITERS):
                g0 = gstart[gi]
                il_t, ih_t = il_ts[gi], ih_ts[gi]
                nlo, nhi = nb * KL, nb * KH
                xg_lo = gp.tile([128, nlo, IN_CH], BF16, tag="lo")
                xg_hi = gp.tile([128, nhi, IN_CH], BF16, tag="hi")
                ea_g, rr_g = ea_gs[gi], rr_gs[gi]
                if "gather" not in skip:
                    # one lo + one hi call per block so each block's
                    # compute gates only on its own slice of the gather
                    for bb in range(nb):
                        c0, cn = bb * KL, KL
                        nc.gpsimd.dma_gather(
                            xg_lo[:, c0:c0 + cn, :], xlo[:, :],
                            il_t[:, c0 * 8:(c0 + cn) * 8],
                            cn * 128, cn * 128, IN_CH, single_packet=False,
                            queue_num=qn % 4)
                        qn += 1
                        c0, cn = bb * KH, KH
                        nc.gpsimd.dma_gather(
                            xg_hi[:, c0:c0 + cn, :], xhi[:, :],
                            ih_t[:, c0 * 8:(c0 + cn) * 8],
                            cn * 128, cn * 128, IN_CH, single_packet=False,
                            queue_num=qn % 4)
                        qn += 1
                if gi + 2 < NGI:
                    prefetch_streams(gi + 2)

                for bb in range(nb):
                    b = g0 + bb
                    ps_x = pagg.tile([IN_CH, BLK], F32, tag="psx")
                    ps_e = pagg.tile([BLK, EDGE_DIM], F32, tag="pse")
                    if len(pse_hist) >= 2:
                        # one DVE wait that dominates all 32 P-tile
                        # anti-deps of this block (P ring = 2 blocks), so
                        # the per-chunk waits are elided
                        nc.vector.tensor_copy(out=dummy_t[:, 0:1],
                                              in_=pse_hist[-2][0:1, 0:1])
                    pse_hist.append(ps_e)
                    for k in range(K):
                        c = bb * K + k
                        P = pb.tile([128, 128], BF16)
                        if "onehot" not in skip:
                            nc.vector.tensor_scalar(
                                out=P[:],
                                in0=iota_t[:],
                                scalar1=rr_g[:, c, 0:1],
                                scalar2=rr_g[:, c, 1:2],
                                op0=mybir.AluOpType.is_equal,
                                op1=mybir.AluOpType.mult,
                            )
                        if k < KL:
                            rhs_x = xg_lo[:, bb * KL + k, :]
                        else:
                            rhs_x = xg_hi[:, bb * KH + (k - KL), :]
                        if "mm" not in skip:
                            nc.tensor.matmul(ps_x[:], lhsT=rhs_x, rhs=P[:],
                                             start=(k == 0), stop=(k == K - 1))
                            nc.tensor.matmul(ps_e[:], lhsT=P[:], rhs=ea_g[:, c, :],
                                             start=(k == 0), stop=(k == K - 1))

                    aggT_x = st.tile([128, BLK], BF16, tag="aggtx")
                    agg_e = st.tile([BLK, EDGE_DIM], BF16, tag="aggsb")
                    nc.scalar.copy(aggT_x[:], ps_x[:])
                    nc.scalar.copy(agg_e[:], ps_e[:])
                    pt2 = pmisc.tile([EDGE_DIM, BLK], BF16, tag="pt2", bufs=2)
                    nc.tensor.transpose(pt2[:], agg_e[:], ident_t[:])
                    aggT_e = st.tile([EDGE_DIM, BLK], BF16, tag="aggte")
                    nc.scalar.copy(aggT_e[:], pt2[:])
                    ps_o = pmisc.tile([128, OUT_CH], F32, tag="pso", bufs=2)
                    nc.tensor.matmul(ps_o[:], lhsT=aggT_x[:], rhs=w1_t[:],
                                     start=True, stop=False)
                    nc.tensor.matmul(ps_o[:], lhsT=aggT_e[:], rhs=w2_t[:],
                                     start=False, stop=False)
                    # bias via rank-1 matmul: ones[1,128]^T @ b[1,128]
                    nc.tensor.matmul(ps_o[:], lhsT=ones_t[:], rhs=b_t[:],
                                     start=False, stop=True)
                    if b % 2 == 0:
                        out_pair[0] = st.tile([128, 2, OUT_CH], BF16, tag="outsb", name="out_pair")
                    nc.scalar.copy(out_pair[0][:, b % 2, :], ps_o[:])
                    if b % 2 == 1:
                        nc.sync.dma_start(out=out[:, b - 1:b + 1, :],
                                          in_=out_pair[0][:, :, :])
                    elif b == NB - 1:
                        nc.sync.dma_start(out=out[:, b:b + 1, :],
                                          in_=out_pair[0][:, 0:1, :])
    nc.finalize()
    return nc


def _wrap16(idx_core):
    """[NB*KX*128] -> [128, NB*KX*8] int16 SWDGE wrapped layout.

    Column j of each 16-partition group holds indices [16j, 16j+16);
    any slice at 16-index granularity is itself well-formed, so one
    flat array serves every per-call slice."""
    n = idx_core.shape[0]
    a = idx_core.reshape(n // 16, 16).T  # [16, n//16]
    return np.ascontiguousarray(np.tile(a, (8, 1)).astype(np.int16))


def _balance_rows(row):
    """Deal rows into NBLOCKS blocks: <=128 rows per block, edge loads as
    even as possible.  Returns block_of_row[N], rrel_of_row[N], max load."""
    deg = np.bincount(row, minlength=N_NODES).astype(np.int64)
    order = np.argsort(-deg, kind="stable")
    loads = np.zeros(NBLOCKS, dtype=np.int64)
    nrows = np.zeros(NBLOCKS, dtype=np.int32)
    block_of_row = np.empty(N_NODES, dtype=np.int32)
    # greedy rounds: biggest remaining rows -> least-loaded blocks.
    # each round hands each block at most one row, so nrows <= 128.
    pos = 0
    while pos < N_NODES:
        nround = min(NBLOCKS, N_NODES - pos)
        rows_r = order[pos:pos + nround]          # degree descending
        border = np.argsort(loads, kind="stable")[:nround]
        block_of_row[rows_r] = border
        loads[border] += deg[rows_r]
        nrows[border] += 1
        pos += nround
    # refinement: move rows off the most-loaded block
    for _ in range(3000):
        bmax = int(np.argmax(loads))
        bmin = int(np.argmin(loads))
        if loads[bmax] - loads[bmin] <= 2 or nrows[bmin] >= 128:
            break
        rows_b = np.flatnonzero(block_of_row == bmax)
        cand = rows_b[deg[rows_b] > 0]
        if cand.size == 0:
            break
        want = (loads[bmax] - loads[bmin]) // 2
        r = cand[int(np.argmin(np.abs(deg[cand] - want)))]
        if deg[r] >= loads[bmax] - loads[bmin]:
            break
        block_of_row[r] = bmin
        loads[bmax] -= deg[r]
        loads[bmin] += deg[r]
        nrows[bmax] -= 1
        nrows[bmin] += 1
    # assign rrel slots within each block
    bsort = np.argsort(block_of_row, kind="stable")
    bo = block_of_row[bsort]
    starts = np.searchsorted(bo, np.arange(NBLOCKS))
    rrel_of_row = np.empty(N_NODES, dtype=np.int32)
    rrel_of_row[bsort] = np.arange(N_NODES) - starts[bo]
    return block_of_row, rrel_of_row, int(loads.max())


def _preprocess(row, col, norm, eattr):
    E = row.shape[0]
    block_of_row, rrel_of_row, maxload = _balance_rows(row)

    KL = KH = 16
    while maxload > (KL + KH) * 128:
        KL += 1
    K = KL + KH

    bid = block_of_row[row]
    rrel = rrel_of_row[row]

    # lo/hi class per edge: 0 = lo-only, 1 = flex, 2 = hi-only
    cls = np.where(col < HI_BASE, 0, np.where(col < SPLIT, 1, 2))

    order = np.argsort(bid * 4 + cls, kind="stable")
    key_s = (bid * 4 + cls)[order]
    grp_start = np.searchsorted(key_s, np.arange(NBLOCKS * 4))
    grp_cnt = np.diff(np.append(grp_start, E)).reshape(NBLOCKS, 4)
    L, Fx, H = grp_cnt[:, 0], grp_cnt[:, 1], grp_cnt[:, 2]
    # flex edges assigned to the lo side: enough that hi fits, at most
    # lo capacity
    take = np.clip(H + Fx - KH * 128, 0, np.minimum(Fx, KL * 128 - L))
    if np.any(L + take > KL * 128) or np.any(H + Fx - take > KH * 128):
        KL += 1
        KH += 1
        K = KL + KH
        take = np.clip(H + Fx - KH * 128, 0, np.minimum(Fx, KL * 128 - L))

    pos_in_grp = np.arange(E) - grp_start[key_s]
    cls_s = key_s & 3
    bid_s = key_s >> 2
    is_lo = (cls_s == 0) | ((cls_s == 1) & (pos_in_grp < take[bid_s]))
    slot_lo = np.where(cls_s == 0, pos_in_grp, L[bid_s] + pos_in_grp)
    slot_hi = np.where(cls_s == 1, pos_in_grp - take[bid_s],
                       (Fx - take)[bid_s] + pos_in_grp)
    slot = np.where(is_lo, slot_lo, KL * 128 + slot_hi)

    core_s = bid_s // NB
    bloc_s = bid_s % NB
    dst = (core_s * NB + bloc_s) * (K * 128) + slot

    S = N_CORES * NB * K * 128
    col_pad = np.zeros(S, dtype=np.int32)
    col_pad.reshape(-1, K * 128)[:, KL * 128:] = HI_BASE
    norm_pad = np.zeros(S, dtype=np.float32)
    rrel_pad = np.zeros(S, dtype=np.float32)
    col_pad[dst] = col[order]
    norm_pad[dst] = norm[order]
    rrel_pad[dst] = rrel[order]
    ea_pad = np.zeros((S, EDGE_DIM), dtype=ml_dtypes.float8_e4m3)
    ea_pad[dst] = eattr[order].astype(ml_dtypes.float8_e4m3)

    # gather index streams, wrapped-16, whole core
    colr = col_pad.reshape(N_CORES, NB, K, 128)
    lo_flat = np.ascontiguousarray(colr[:, :, :KL, :]).reshape(N_CORES, -1)
    hi_flat = np.ascontiguousarray(colr[:, :, KL:, :] - HI_BASE).reshape(N_CORES, -1)
    idxlo = [_wrap16(lo_flat[c]) for c in range(N_CORES)]
    idxhi = [_wrap16(hi_flat[c]) for c in range(N_CORES)]

    # partition-major streams: edge (block b, chunk k, partition p)
    # -> [core, p, b*K+k, :]
    ea4 = ea_pad.reshape(N_CORES, NB * K, 128, EDGE_DIM)
    ea_h = np.ascontiguousarray(ea4.transpose(0, 2, 1, 3))
    rr2 = np.stack([rrel_pad, norm_pad], axis=1)
    rr4 = rr2.reshape(N_CORES, NB * K, 128, 2)
    rr_h = np.ascontiguousarray(rr4.transpose(0, 2, 1, 3))
    return KL, KH, idxlo, idxhi, ea_h, rr_h, block_of_row, rrel_of_row


def _run_device(x, row, col, norm, eattr, W, b):
    from concourse import bass_utils

    (KL, KH, idxlo, idxhi, ea_h, rr_h,
     block_of_row, rrel_of_row) = _preprocess(row, col, norm, eattr)
    key = (KL, KH)
    if key not in _NC_CACHE:
        _NC_CACHE.clear()
        _NC_CACHE[key] = _build_nc(KL, KH)
    nc = _NC_CACHE[key]

    x_bf = _to_bf16(x)
    xlo = np.ascontiguousarray(x_bf[:SPLIT])
    xhi = np.ascontiguousarray(x_bf[HI_BASE:])
    iota_h = np.tile(
        np.arange(128, dtype=np.float32).astype(ml_dtypes.bfloat16)[None, :],
        (128, 1))
    ident_h = np.eye(128, dtype=np.float32).astype(ml_dtypes.bfloat16)
    W_bf = _to_bf16(W)
    b_h = _to_bf16(np.asarray(b, dtype=np.float32))[None, :]
    ones_h = np.ones((1, 128), dtype=np.float32).astype(ml_dtypes.bfloat16)

    in_maps = []
    for c in range(N_CORES):
        in_maps.append({
            "xlo": xlo, "xhi": xhi,
            "idxlo": idxlo[c], "idxhi": idxhi[c],
            "ea_h": ea_h[c], "rr_h": rr_h[c],
            "iota_h": iota_h, "ident_h": ident_h,
            "W_h": W_bf, "b_h": b_h, "ones_h": ones_h,
        })
    res = bass_utils.run_bass_kernel_spmd(nc, in_maps,
                                          core_ids=list(range(N_CORES)))
    allout = np.stack([np.asarray(res.results[i]["out"], dtype=np.float32)
                       for i in range(N_CORES)], axis=0)  # [8, 128, NB, 128]
    # un-permute: row r lives at (core, partition rrel, local block)
    core_of_row = block_of_row // NB
    bloc_of_row = block_of_row % NB
    return np.ascontiguousarray(
        allout[core_of_row, rrel_of_row, bloc_of_row])


def _segment_sum(msg, row, n):
    order = np.argsort(row, kind="stable")
    rs = row[order]
    ms = msg[order]
    starts = np.concatenate(([0], np.flatnonzero(np.diff(rs)) + 1))
    sums = np.add.reduceat(ms, starts, axis=0)
    out = np.zeros((n, msg.shape[1]), dtype=msg.dtype)
    out[rs[starts]] = sums
    return out


def _cpu_fallback(x, row, col, norm, eattr, W, b):
    msg = np.empty((N_EDGES, F), dtype=np.float32)
    np.multiply(x[col], norm[:, None], out=msg[:, :IN_CH])
    np.multiply(eattr, norm[:, None], out=msg[:, IN_CH:])
    agg = _segment_sum(msg, row, N_NODES)
    return (agg @ W + b[None, :]).astype(np.float32)


def kernel(**inputs) -> np.ndarray:
    x = np.ascontiguousarray(inputs["x"], dtype=np.float32)
    row = np.ascontiguousarray(inputs["row"]).astype(np.int64)
    col = np.ascontiguousarray(inputs["col"]).astype(np.int64)
    norm = np.ascontiguousarray(inputs["norm"], dtype=np.float32)
    eattr = np.ascontiguousarray(inputs["edge_attr"], dtype=np.float32)
    W = np.ascontiguousarray(inputs["W"], dtype=np.float32)
    b = np.ascontiguousarray(inputs["b"], dtype=np.float32)
    try:
        return _run_device(x, row, col, norm, eattr, W, b)
    except Exception:
        import traceback
        traceback.print_exc()
        return _cpu_fallback(x, row, col, norm, eattr, W, b)
